# revision 14
# baseline (speedup 1.0000x reference)
"""Trainium2 Bass kernel for nn_Block_38053410242840 (dense transformer block).

Strategy: data-parallel over batch (B=8 -> 8 NeuronCores, zero collectives).
Per core, one batch element [T=1024, C=1024] flows feature-major
(activations stored [feature partitions, token free]) so every matmul's
contraction dim sits on SBUF partitions with no on-device transposes:
the host pre-transposes weights/x and pre-casts weights to bf16.

v2 schedule: LN1 -> v-proj -> kq pairs interleaved with attention (the
softmax exp/ACT work of quarter q overlaps the kq matmuls of quarter
q+1) -> proj with LN2 stats interleaved -> fc1+gelu -> fc2.
Engine balance: kq evicts on DVE, LN elementwise split DVE/GpSimd,
z-row copies split ACT/DVE, all broadcast matmul operands bf16.
"""
import sys

sys.path.insert(0, "/opt/trn_rl_repo")

from contextlib import ExitStack

import ml_dtypes
import numpy as np

import concourse.bass as bass
import concourse.tile as tile
from concourse import bacc, mybir
from concourse import bass_utils

F32 = mybir.dt.float32
BF16 = mybir.dt.bfloat16
AF = mybir.ActivationFunctionType
ALU = mybir.AluOpType
ts = bass.ts

P = 128
T = 1024
C = 1024
H = 16
HD = 64
LN_EPS = 1e-5
NB = 8  # cores / batch


def act_raw(nc, out, in_, func, bias=0.0, scale=1.0):
    """InstActivation with immediate bias/scale (bypasses the Reciprocal
    accuracy guard; HW-measured max-rel 1.2e-5 on [1, 2000])."""
    eng = nc.scalar
    inputs = [eng.lower_ap(in_)]
    for arg in (bias, scale, 0.0):
        inputs.append(mybir.ImmediateValue(dtype=mybir.dt.float32, value=arg))
    return eng.add_instruction(
        mybir.InstActivation(
            name=nc.get_next_instruction_name(),
            func=func,
            ins=inputs,
            outs=[eng.lower_ap(out)],
        )
    )


def build_nc(debug=False):
    nc = bacc.Bacc("TRN2", target_bir_lowering=False, debug=False,
                   enable_asserts=False, num_devices=NB)

    d_xT = nc.dram_tensor("xT", [C, T], F32, kind="ExternalInput").ap()
    d_wkq = nc.dram_tensor("wkq", [C, 2048], BF16, kind="ExternalInput").ap()
    d_wv = nc.dram_tensor("wv", [C, 1024], BF16, kind="ExternalInput").ap()
    d_pw = nc.dram_tensor("pw", [C, 1024], BF16, kind="ExternalInput").ap()
    d_w1 = nc.dram_tensor("w1", [C, 4096], BF16, kind="ExternalInput").ap()
    d_w2 = nc.dram_tensor("w2", [4096, 1024], BF16, kind="ExternalInput").ap()
    # packed f32 consts: [:,0:128]=ones, 128:144 kq bias, 144:152 proj bias(+pb),
    # 152:184 fc1 bias, 184:192 fc2 bias
    d_cfb = nc.dram_tensor("cfb", [P, 200], F32, kind="ExternalInput").ap()
    # packed bf16 consts: [:,0:1024]=causal diag masks (2x512), col 1023 ones
    d_mo = nc.dram_tensor("mo", [P, 1152], BF16, kind="ExternalInput").ap()
    d_out = nc.dram_tensor("out", [C, T], F32, kind="ExternalOutput").ap()

    dbg = {}
    if debug:
        dbg["h1"] = nc.dram_tensor("dbg_h1", [P, 8192], BF16, kind="ExternalOutput").ap()
        dbg["k"] = nc.dram_tensor("dbg_k", [P, 8192], BF16, kind="ExternalOutput").ap()
        dbg["q"] = nc.dram_tensor("dbg_q", [P, 8192], BF16, kind="ExternalOutput").ap()
        dbg["v"] = nc.dram_tensor("dbg_v", [P, 8320], BF16, kind="ExternalOutput").ap()
        dbg["y"] = nc.dram_tensor("dbg_y", [P, 8192], BF16, kind="ExternalOutput").ap()
        dbg["x2"] = nc.dram_tensor("dbg_x2", [P, 8192], F32, kind="ExternalOutput").ap()
        dbg["g"] = nc.dram_tensor("dbg_g", [P, 32768], BF16, kind="ExternalOutput").ap()

    with tile.TileContext(nc) as tc:
        with ExitStack() as ctx:
            build_body(ctx, tc, nc, d_xT, d_wkq, d_wv, d_pw, d_w1, d_w2,
                       d_cfb, d_mo, d_out, dbg)
    nc.compile()
    return nc


def build_body(ctx, tc, nc, d_xT, d_wkq, d_wv, d_pw, d_w1, d_w2, d_cfb, d_mo,
               d_out, dbg):
    wp = ctx.enter_context(tc.tile_pool(name="wp", bufs=2))
    lnp = ctx.enter_context(tc.tile_pool(name="lnp", bufs=2))
    tmpp = ctx.enter_context(tc.tile_pool(name="tmpp", bufs=2))
    outp = ctx.enter_context(tc.tile_pool(name="outp", bufs=2))
    zpool = ctx.enter_context(tc.tile_pool(name="zpool", bufs=2))
    # PSUM: psA = [128,1024] double-bank tiles (2 bufs = 4 banks), psB =
    # [65,512]/[33,512] single-bank tiles (4 bufs = 4 banks). psB at depth 4
    # lets two attention (head,chunk) py-pairs coexist so the next pair's
    # att@v accumulation no longer waits on the previous pair's evictions.
    psA = ctx.enter_context(tc.tile_pool(name="psA", bufs=2, space="PSUM"))
    psB = ctx.enter_context(tc.tile_pool(name="psB", bufs=4, space="PSUM"))

    def pa():
        return psA.tile([P, 1024], F32, tag="a", name="pa")

    def pb(part=65):
        return psB.tile([part, 512], F32, tag="b", name="pb")

    # ---- constants ----
    cfb, free_cfb = tc.tile([P, 200], F32, name="cfb_t")
    nc.sync.dma_start(cfb[:], d_cfb[:])
    mo, free_mo = tc.tile([P, 1152], BF16, name="mo_t")
    nc.sync.dma_start(mo[:], d_mo[:])
    ones_f = cfb[:, 0:128]
    kqb = cfb[:, 128:144]
    pbc = cfb[:, 144:152]
    b1c = cfb[:, 152:184]
    b2c = cfb[:, 184:192]
    eps_c = cfb[:, 192:193]
    zero_c = cfb[:, 193:194]
    masks = mo[:, 0:1024]  # diagonal-block mask, doubled
    ones_b = mo[:, 1023:1024]  # mask col 511 copy = all-ones column
    invC_row = mo[0:1, 1024:1152]  # [1,128] row of 1/C

    def ones_row_bf(a, w=64):
        # [1, w] bf16 all-ones at partition base 32a (mask rows are ones
        # for cols >= partition index, so the 512-w..512 window is all-ones
        # for rows 0/32/64)
        return mo[32 * a:32 * a + 1, 512 - w:512]

    # ---- allocation stack (LIFO lifetimes) ----
    x2_all, free_x2 = tc.tile([P, 8192], F32, name="x2_all")
    y_all, free_y = tc.tile([P, 8192], BF16, name="y_all")
    h1, free_h1 = tc.tile([P, 8192], BF16, name="h1")

    # ---- layernorm helpers ----
    def ln_eng(i):
        # ~2:1 split DVE:GpSimd (DVE is ~2x faster at elementwise)
        return nc.gpsimd if i in (2, 4, 6) else nc.vector

    def ln_stats_tile(psS, i, src):
        """Accumulate Σx (f32 mm) and Σx² (bf16 mm) for c-tile i."""
        sq = lnp.tile([P, 1024], BF16, tag="sq", name="sq")
        ln_eng(i).tensor_mul(sq[:], src, src)
        for c in range(2):
            nc.tensor.matmul(psS[c][0:1, :], lhsT=cfb[:, 0:1],
                             rhs=src[:, ts(c, 512)],
                             start=(i == 0), stop=(i == 7))
            nc.tensor.matmul(psS[c][32:33, :], lhsT=ones_b[:, 0:1],
                             rhs=sq[:, ts(c, 512)], start=(i == 0), stop=(i == 7),
                             tile_position=(0, 32))

    def ln_tail(psS, hname):
        """Broadcast-first LN tail: Σx/Σx² rows -> (1/C)-scaled 128-row
        broadcasts -> full-width mean/var/rsqrt (no single-lane ladder)."""
        bc, free_bc = tc.tile([P, 2048], F32, name=hname + "_bc")
        srows, free_srows = tc.tile([1, 2048], BF16, name=hname + "_srows")
        nc.scalar.copy(srows[0:1, 0:512], psS[0][0:1, :])
        nc.scalar.copy(srows[0:1, 512:1024], psS[1][0:1, :])
        nc.scalar.copy(srows[0:1, 1024:1536], psS[0][32:33, :])
        nc.scalar.copy(srows[0:1, 1536:2048], psS[1][32:33, :])
        pm = pa()
        pxx = pa()
        for c in range(2):
            nc.tensor.matmul(pm[:, ts(c, 512)], lhsT=invC_row,
                             rhs=srows[0:1, ts(c, 512)], start=True, stop=True)
            nc.tensor.matmul(pxx[:, ts(c, 512)], lhsT=invC_row,
                             rhs=srows[0:1, 1024 + 512 * c:1536 + 512 * c],
                             start=True, stop=True)
        t2 = tmpp.tile([P, 1024], F32, tag="t1", name=hname + "_t2")
        t3 = tmpp.tile([P, 1024], F32, tag="t1", name=hname + "_t3")
        nc.scalar.copy(bc[:, 0:1024], pm[:])
        nc.vector.tensor_mul(t2[:], bc[:, 0:1024], bc[:, 0:1024])
        nc.vector.scalar_tensor_tensor(t3[:], t2[:], -1.0, pxx[:],
                                       ALU.mult, ALU.add)
        act_raw(nc, bc[:, 1024:2048], t3[:], AF.Rsqrt, bias=LN_EPS)
        free_srows()
        return bc, free_bc

    def ln_norm_tile(bc, i, src, dst):
        eng = ln_eng(i)
        t1 = tmpp.tile([P, 1024], F32, tag="t1", name="t1")
        eng.tensor_sub(t1[:], src, bc[:, 0:1024])
        eng.tensor_mul(dst, t1[:], bc[:, 1024:2048])

    # ---- LN1 (x cached in SBUF across stats+normalize) ----
    lnx, free_lnx = tc.tile([P, 8192], F32, name="lnx")
    for i in range(8):
        nc.sync.dma_start(lnx[:, ts(i, 1024)], d_xT[ts(i, 128), :])
    # PE warm-up: dummy matmul burst into a scratch psum during the x-DMA
    # wait so the HAM clock gate reaches 8/8 before the real work starts
    pw_ = pa()
    for _w in range(10):
        nc.tensor.matmul(pw_[:, 0:512], lhsT=ones_b[:, 0:1].broadcast(1, 128)
                         if False else mo[:, 0:128],
                         rhs=mo[:, 0:512], start=(_w == 0), stop=(_w == 9))
    psS1 = [pb(33) for _ in range(2)]
    for i in range(8):
        ln_stats_tile(psS1, i, lnx[:, ts(i, 1024)])
    bc1, free_bc1 = ln_tail(psS1, "h1")
    for i in range(8):
        ln_norm_tile(bc1, i, lnx[:, ts(i, 1024)], h1[:, ts(i, 1024)])
    free_bc1()
    free_lnx()
    if dbg:
        nc.sync.dma_start(dbg["h1"][:], h1[:])

    # ---- v projection (token-major, fused ones column per head) ----
    k_all, free_k = tc.tile([P, 8192], BF16, name="k_all")
    q_all, free_q = tc.tile([P, 8192], BF16, name="q_all")
    v_all, free_v = tc.tile([P, 8320], BF16, name="v_all")
    wv_all, free_wv = tc.tile([P, 8192], BF16, name="wv_all")
    wv_v = d_wv.rearrange("(ct p) o -> p ct o", p=128)
    for c in range(8):
        nc.sync.dma_start(wv_all[:, ts(c, 1024)], wv_v[:, c, :])
    v_view = v_all[:].rearrange("p (a c) -> p a c", c=65)
    nc.vector.memset(v_view[:, :, 64:65], 1.0)
    v_hview = v_all[:].rearrange("p (jt h c) -> p jt h c", jt=8, c=65)
    for jt in range(8):
        psv = pa()
        for c in range(8):
            lhs = h1[:, 1024 * c + 128 * jt:1024 * c + 128 * jt + 128]
            for half in range(2):
                nc.tensor.matmul(psv[:, ts(half, 512)], lhsT=lhs,
                                 rhs=wv_all[:, 1024 * c + 512 * half:1024 * c + 512 * half + 512],
                                 start=(c == 0), stop=(c == 7))
        nc.scalar.copy(v_hview[:, jt, :, 0:64],
                       psv[:].rearrange("p (h c) -> p h c", c=64))
    free_wv()

    # ---- attention helpers ----
    y2_all, free_y2 = tc.tile([P, 8192], BF16, name="y2_all")
    e_buf, free_e = tc.tile([P, 4096], BF16, name="e_buf")
    e_rot = [0]

    def e_slot():
        i = e_rot[0] % 4
        e_rot[0] += 1
        return e_buf[:, 1024 * i:1024 * i + 1024]

    def normalize_quarter(q4, zstash):
        # y2 = y' * (1/Z) broadcast; two broadcasts share one psA tile
        pzt = [None]

        def pz_half(idx):
            if idx % 2 == 0:
                pzt[0] = pa()
            return pzt[0][0:64, 512 * (idx % 2):512 * (idx % 2) + 512]

        idx = 0
        zi_t = tmpp.tile([65, 1536], BF16, tag="zi", name="zi")
        act_raw(nc, zi_t[:], zstash[:], AF.Reciprocal)
        for b in range(3):
            zi = zi_t[:, 512 * b:512 * b + 512]
            for a in range(3):
                li = 3 * b + a
                if li >= 8:
                    break
                hh = 4 * q4 + li // 2
                ch = li % 2
                j, m2 = hh // 2, hh % 2
                r = 64 * m2
                col = 1024 * j + 512 * ch
                pz = pz_half(idx)
                idx += 1
                nc.tensor.matmul(pz, lhsT=ones_row_bf(a),
                                 rhs=zi[32 * a:32 * a + 1, :], start=True, stop=True)
                nc.vector.tensor_mul(y2_all[r:r + 64, col:col + 512], pz,
                                     y_all[r:r + 64, col:col + 512])

    zstashes = {}

    def attn_quarter(q4):
        # 8 Z-row slots per quarter: partition base 32a, col block 512b
        zstash = zpool.tile([65, 1536], F32, tag="zs", name=f"zstash{q4}")
        # only rows 0/32/64 carry Z; zero the rest so the batched reciprocal
        # reads initialized memory (CoreSim requirement, free on GpSimd)
        nc.gpsimd.memset(zstash[:], 1.0)
        zstashes[q4] = zstash
        for j in (2 * q4, 2 * q4 + 1):
            for ch in range(2):
                ntk = 4 if ch == 0 else 8
                py = [pb(), pb()]
                qcol = 1024 * j + 512 * ch
                for jt in range(ntk):
                    pcol = 1024 * j + 128 * jt
                    m = jt - 4 * ch
                    # diagonal trim: tq columns < 128*m are fully masked; skip
                    o = 128 * m if m > 0 else 0
                    ps_ = pa()
                    for m2 in range(2):
                        r = 64 * m2
                        nc.tensor.matmul(ps_[:, 512 * m2 + o:512 * m2 + 512],
                                         lhsT=k_all[r:r + 64, pcol:pcol + 128],
                                         rhs=q_all[r:r + 64, qcol + o:qcol + 512],
                                         start=True, stop=True)
                    et_t = e_slot()
                    et = et_t.rearrange("p (h c) -> p h c", c=512)
                    ps_v = ps_[:].rearrange("p (h c) -> p h c", c=512)
                    nc.scalar.activation(et[:, :, o:512], ps_v[:, :, o:512],
                                         AF.Exp, bias=zero_c, scale=0.125)
                    if m >= 0:
                        # in-place diagonal-block mask (exact-alias DVE, probed safe)
                        nc.vector.tensor_mul(
                            et[:, :, o:o + 128], et[:, :, o:o + 128],
                            masks[:].rearrange("p (h c) -> p h c", c=512)[:, 0:2, 0:128])
                    for m2 in range(2):
                        hh = 2 * j + m2
                        nc.tensor.matmul(
                            py[m2][:, o:512],
                            lhsT=v_all[:, 1040 * jt + 65 * hh:1040 * jt + 65 * hh + 65],
                            rhs=et[:, m2, o:512],
                            start=(jt == 0), stop=(jt == ntk - 1))
                for m2 in range(2):
                    hh = 2 * j + m2
                    r = 64 * m2
                    col = 1024 * j + 512 * ch
                    li = (hh % 4) * 2 + ch
                    a, b = li % 3, li // 3
                    if m2 == 0:
                        nc.scalar.copy(y_all[r:r + 64, col:col + 512],
                                       py[m2][0:64, :])
                    else:
                        nc.vector.tensor_copy(y_all[r:r + 64, col:col + 512],
                                              py[m2][0:64, :])
                    zdst = zstash[32 * a:32 * a + 1, 512 * b:512 * b + 512]
                    nc.vector.tensor_copy(zdst, py[m2][64:65, :])

    # ---- kq pairs interleaved with attention quarters ----
    wkq_v = d_wkq.rearrange("(ct p) o -> p ct o", p=128)
    for g4 in range(4):
        wg = wp.tile([P, 4096], BF16, tag="wg", name="wg")
        wgv = wg[:].rearrange("p (ct o) -> p ct o", o=512)
        for cc in range(4):
            nc.sync.dma_start(wgv[:, 2 * cc:2 * cc + 2, :],
                              wkq_v[:, 2 * cc:2 * cc + 2, ts(g4, 512)])
        for hl in range(4):
            hh = 4 * g4 + hl
            j, r = hh // 2, (hh % 2) * 64
            pp = pa()
            for c in range(8):
                for ch in range(2):
                    nc.tensor.matmul(pp[:, ts(ch, 512)], lhsT=wgv[:, c, ts(hl, 128)],
                                     rhs=h1[:, 1024 * c + 512 * ch:1024 * c + 512 * ch + 512],
                                     start=(c == 0), stop=(c == 7))
            col = 1024 * j
            nc.vector.tensor_scalar_add(k_all[r:r + 64, col:col + 1024], pp[0:64, :],
                                        kqb[0:64, hh:hh + 1])
            nc.vector.tensor_scalar_add(q_all[r:r + 64, col:col + 1024], pp[64:128, :],
                                        kqb[64:128, hh:hh + 1])
        if g4 > 0:
            attn_quarter(g4 - 1)
        if g4 > 1:
            normalize_quarter(g4 - 2, zstashes.pop(g4 - 2))
    attn_quarter(3)
    normalize_quarter(2, zstashes.pop(2))
    normalize_quarter(3, zstashes.pop(3))
    free_e()
    if dbg:
        nc.sync.dma_start(dbg["k"][:], k_all[:])
        nc.sync.dma_start(dbg["q"][:], q_all[:])
        nc.sync.dma_start(dbg["v"][:], v_all[:])
        nc.sync.dma_start(dbg["y"][:], y2_all[:])

    # ---- proj + residual, LN2 stats interleaved ----
    psS2 = [pb(33) for _ in range(2)]
    pw_v = d_pw.rearrange("(ct p) o -> p ct o", p=128)
    for jg in range(2):
        wg = wp.tile([P, 4096], BF16, tag="wg", name="wgp")
        wgv = wg[:].rearrange("p (ct o) -> p ct o", o=512)
        for cc in range(4):
            nc.sync.dma_start(wgv[:, 2 * cc:2 * cc + 2, :],
                              pw_v[:, 2 * cc:2 * cc + 2, ts(jg, 512)])
        for jl in range(4):
            jj = 4 * jg + jl
            pp = pa()
            for c in range(8):
                for ch in range(2):
                    nc.tensor.matmul(pp[:, ts(ch, 512)], lhsT=wgv[:, c, ts(jl, 128)],
                                     rhs=y2_all[:, 1024 * c + 512 * ch:1024 * c + 512 * ch + 512],
                                     start=(c == 0), stop=(c == 7))
            xr = tmpp.tile([P, 1024], F32, tag="xs", name="xr")
            nc.sync.dma_start(xr[:], d_xT[ts(jj, 128), :])
            nc.vector.scalar_tensor_tensor(
                x2_all[:, ts(jj, 1024)], pp[:], pbc[:, jj:jj + 1],
                xr[:], ALU.add, ALU.add)
            ln_stats_tile(psS2, jj, x2_all[:, ts(jj, 1024)])
    free_y2()
    free_v()
    free_q()
    free_k()
    free_h1()
    free_y()
    if dbg:
        nc.sync.dma_start(dbg["x2"][:], x2_all[:])

    # ---- LN2 tail + fc1 + gelu (g allocated below h2 so h2 frees first) ----
    g_all, free_g = tc.tile([P, 32768], BF16, name="g_all")
    h2, free_h2 = tc.tile([P, 8192], BF16, name="h2")
    bc2, free_bc2 = ln_tail(psS2, "h2")
    for i in range(8):
        ln_norm_tile(bc2, i, x2_all[:, ts(i, 1024)], h2[:, ts(i, 1024)])
    free_bc2()
    w1_v = d_w1.rearrange("(ct p) o -> p ct o", p=128)
    for og in range(8):
        wg = wp.tile([P, 4096], BF16, tag="wg", name="wg1")
        wgv = wg[:].rearrange("p (ct o) -> p ct o", o=512)
        for cc in range(4):
            nc.sync.dma_start(wgv[:, 2 * cc:2 * cc + 2, :],
                              w1_v[:, 2 * cc:2 * cc + 2, ts(og, 512)])
        for ol in range(4):
            oo = 4 * og + ol
            pp = pa()
            for c in range(8):
                for ch in range(2):
                    nc.tensor.matmul(pp[:, ts(ch, 512)], lhsT=wgv[:, c, ts(ol, 128)],
                                     rhs=h2[:, 1024 * c + 512 * ch:1024 * c + 512 * ch + 512],
                                     start=(c == 0), stop=(c == 7))
            nc.scalar.activation(g_all[:, ts(oo, 1024)], pp[:],
                                 AF.Gelu, bias=b1c[:, oo:oo + 1])
    free_h2()
    if dbg:
        nc.sync.dma_start(dbg["g"][:], g_all[:])

    # ---- fc2 + residual -> out ----
    w2_v = d_w2.rearrange("(kk p) o -> p kk o", p=128)
    for j in range(8):
        wg = wp.tile([P, 4096], BF16, tag="wg", name="wg2")
        wgv = wg[:].rearrange("p (kk o) -> p kk o", o=128)
        for kg in range(4):
            nc.sync.dma_start(wgv[:, 8 * kg:8 * kg + 8, :],
                              w2_v[:, 8 * kg:8 * kg + 8, ts(j, 128)])
        pp = pa()
        for kk in range(32):
            for ch in range(2):
                nc.tensor.matmul(pp[:, ts(ch, 512)], lhsT=wgv[:, kk, :],
                                 rhs=g_all[:, 1024 * kk + 512 * ch:1024 * kk + 512 * ch + 512],
                                 start=(kk == 0), stop=(kk == 31))
        x3 = outp.tile([P, 1024], F32, tag="x3", name="x3")
        nc.vector.scalar_tensor_tensor(
            x3[:], pp[:], b2c[:, j:j + 1],
            x2_all[:, ts(j, 1024)], ALU.add, ALU.add)
        nc.sync.dma_start(d_out[ts(j, 128), :], x3[:])
    free_g()
    free_x2()
    free_mo()
    free_cfb()


# ---------------- host side ----------------

def prep_inputs(inputs):
    """Build the per-core in_maps from the full problem inputs."""
    f32 = np.float32
    bf16 = ml_dtypes.bfloat16
    x = np.asarray(inputs["x"], f32)
    kqv_w = np.asarray(inputs["kqv_w"], f32)
    kqv_b = np.asarray(inputs["kqv_b"], f32)
    proj_w = np.asarray(inputs["proj_w"], f32)
    proj_b = np.asarray(inputs["proj_b"], f32)
    fc1_w = np.asarray(inputs["fc1_w"], f32)
    fc1_b = np.asarray(inputs["fc1_b"], f32)
    fc2_w = np.asarray(inputs["fc2_w"], f32)
    fc2_b = np.asarray(inputs["fc2_b"], f32)

    wT = np.ascontiguousarray(kqv_w.T).reshape(C, H, 192)
    wkq = np.ascontiguousarray(wT[:, :, :128].reshape(C, 2048)).astype(bf16)
    wv = np.ascontiguousarray(wT[:, :, 128:].reshape(C, 1024)).astype(bf16)
    pw = np.ascontiguousarray(proj_w.T).astype(bf16)
    w1 = np.ascontiguousarray(fc1_w.T).astype(bf16)
    w2 = np.ascontiguousarray(fc2_w.T).astype(bf16)

    kq_b = kqv_b.reshape(H, 192)[:, :128].T  # [128, 16]
    v_b = kqv_b.reshape(H, 192)[:, 128:].reshape(C)
    pb = proj_b + proj_w.astype(np.float64) @ v_b.astype(np.float64)
    pb_col = pb.astype(f32).reshape(8, 128).T  # [128, 8]
    b1_col = fc1_b.reshape(32, 128).T  # [128, 32]
    b2_col = fc2_b.reshape(8, 128).T  # [128, 8]

    cfb = np.zeros((P, 200), f32)
    cfb[:, 0:128] = 1.0
    cfb[:, 128:144] = kq_b
    cfb[:, 144:152] = pb_col
    cfb[:, 152:184] = b1_col
    cfb[:, 184:192] = b2_col
    cfb[:, 192] = LN_EPS

    mo = np.zeros((P, 1024), np.float32)
    pcol = np.arange(128)[:, None]
    frow = np.arange(512)[None, :]
    blk = (frow >= pcol).astype(np.float32)
    mo[:, 0:512] = blk
    mo[:, 512:1024] = blk
    mo = np.concatenate([mo, np.full((P, 128), 1.0 / C, np.float32)], axis=1)
    mo = mo.astype(bf16)

    xT = np.ascontiguousarray(x.transpose(0, 2, 1)).astype(f32)  # [B, C, T]

    shared = dict(wkq=wkq, wv=wv, pw=pw, w1=w1, w2=w2, cfb=cfb, mo=mo)
    in_maps = [dict(shared, xT=xT[b]) for b in range(NB)]
    return in_maps


_CACHE = {}


def get_nc(debug=False):
    key = bool(debug)
    if key not in _CACHE:
        _CACHE[key] = build_nc(debug=debug)
    return _CACHE[key]


def run(inputs, debug=False, trace=False):
    nc = get_nc(debug=debug)
    in_maps = prep_inputs(inputs)
    res = bass_utils.run_bass_kernel_spmd(nc, in_maps, core_ids=list(range(NB)),
                                          trace=trace)
    return res


def kernel(**inputs):
    res = run(inputs, debug=False, trace=False)
    out = np.stack([np.asarray(res.results[b]["out"]).T for b in range(NB)])
    return np.ascontiguousarray(out.astype(np.float32))


# revision 17
# speedup vs baseline: 1.0490x; 1.0490x over previous
"""Trainium2 Bass kernel for nn_Block_38053410242840 (dense transformer block).

Strategy: data-parallel over batch (B=8 -> 8 NeuronCores, zero collectives).
Per core, one batch element [T=1024, C=1024] flows feature-major
(activations stored [feature partitions, token free]) so every matmul's
contraction dim sits on SBUF partitions with no on-device transposes:
the host pre-transposes weights/x and pre-casts weights to bf16.

v2 schedule: LN1 -> v-proj -> kq pairs interleaved with attention (the
softmax exp/ACT work of quarter q overlaps the kq matmuls of quarter
q+1) -> proj with LN2 stats interleaved -> fc1+gelu -> fc2.
Engine balance: kq evicts on DVE, LN elementwise split DVE/GpSimd,
z-row copies split ACT/DVE, all broadcast matmul operands bf16.
"""
import sys

sys.path.insert(0, "/opt/trn_rl_repo")

from contextlib import ExitStack

import ml_dtypes
import numpy as np

import concourse.bass as bass
import concourse.tile as tile
from concourse import bacc, mybir
from concourse import bass_utils

F32 = mybir.dt.float32
BF16 = mybir.dt.bfloat16
AF = mybir.ActivationFunctionType
ALU = mybir.AluOpType
ts = bass.ts

P = 128
T = 1024
C = 1024
H = 16
HD = 64
LN_EPS = 1e-5
NB = 8  # cores / batch


def act_raw(nc, out, in_, func, bias=0.0, scale=1.0):
    """InstActivation with immediate bias/scale (bypasses the Reciprocal
    accuracy guard; HW-measured max-rel 1.2e-5 on [1, 2000])."""
    eng = nc.scalar
    inputs = [eng.lower_ap(in_)]
    for arg in (bias, scale, 0.0):
        inputs.append(mybir.ImmediateValue(dtype=mybir.dt.float32, value=arg))
    return eng.add_instruction(
        mybir.InstActivation(
            name=nc.get_next_instruction_name(),
            func=func,
            ins=inputs,
            outs=[eng.lower_ap(out)],
        )
    )


def build_nc(debug=False):
    nc = bacc.Bacc("TRN2", target_bir_lowering=False, debug=False,
                   enable_asserts=False, num_devices=NB)

    d_xT = nc.dram_tensor("xT", [C, T], F32, kind="ExternalInput").ap()
    d_wkq = nc.dram_tensor("wkq", [C, 2048], BF16, kind="ExternalInput").ap()
    d_wv = nc.dram_tensor("wv", [C, 1024], BF16, kind="ExternalInput").ap()
    d_pw = nc.dram_tensor("pw", [C, 1024], BF16, kind="ExternalInput").ap()
    d_w1 = nc.dram_tensor("w1", [C, 4096], BF16, kind="ExternalInput").ap()
    d_w2 = nc.dram_tensor("w2", [4096, 1024], BF16, kind="ExternalInput").ap()
    # packed f32 consts: [:,0:128]=ones, 128:144 kq bias, 144:152 proj bias(+pb),
    # 152:184 fc1 bias, 184:192 fc2 bias
    d_cfb = nc.dram_tensor("cfb", [P, 200], F32, kind="ExternalInput").ap()
    # packed bf16 consts: [:,0:1024]=causal diag masks (2x512), col 1023 ones
    d_mo = nc.dram_tensor("mo", [P, 1152], BF16, kind="ExternalInput").ap()
    d_out = nc.dram_tensor("out", [C, T], F32, kind="ExternalOutput").ap()

    dbg = {}
    if debug:
        dbg["h1"] = nc.dram_tensor("dbg_h1", [P, 8192], BF16, kind="ExternalOutput").ap()
        dbg["k"] = nc.dram_tensor("dbg_k", [P, 8192], BF16, kind="ExternalOutput").ap()
        dbg["q"] = nc.dram_tensor("dbg_q", [P, 8192], BF16, kind="ExternalOutput").ap()
        dbg["v"] = nc.dram_tensor("dbg_v", [P, 8320], BF16, kind="ExternalOutput").ap()
        dbg["y"] = nc.dram_tensor("dbg_y", [P, 8192], BF16, kind="ExternalOutput").ap()
        dbg["x2"] = nc.dram_tensor("dbg_x2", [P, 8192], F32, kind="ExternalOutput").ap()
        dbg["g"] = nc.dram_tensor("dbg_g", [P, 32768], BF16, kind="ExternalOutput").ap()

    with tile.TileContext(nc) as tc:
        with ExitStack() as ctx:
            build_body(ctx, tc, nc, d_xT, d_wkq, d_wv, d_pw, d_w1, d_w2,
                       d_cfb, d_mo, d_out, dbg)
    nc.compile()
    return nc


def build_body(ctx, tc, nc, d_xT, d_wkq, d_wv, d_pw, d_w1, d_w2, d_cfb, d_mo,
               d_out, dbg):
    wp = ctx.enter_context(tc.tile_pool(name="wp", bufs=2))
    lnp = ctx.enter_context(tc.tile_pool(name="lnp", bufs=2))
    tmpp = ctx.enter_context(tc.tile_pool(name="tmpp", bufs=2))
    outp = ctx.enter_context(tc.tile_pool(name="outp", bufs=2))
    zpool = ctx.enter_context(tc.tile_pool(name="zpool", bufs=2))
    # PSUM: psA = [128,1024] double-bank tiles (3 bufs = 6 banks), psB = two
    # [65,512]/[33,512] single-bank tiles (2 banks).
    psA = ctx.enter_context(tc.tile_pool(name="psA", bufs=3, space="PSUM"))
    psB = ctx.enter_context(tc.tile_pool(name="psB", bufs=2, space="PSUM"))

    def pa():
        return psA.tile([P, 1024], F32, tag="a", name="pa")

    def pb(part=65):
        return psB.tile([part, 512], F32, tag="b", name="pb")

    # ---- constants ----
    cfb, free_cfb = tc.tile([P, 200], F32, name="cfb_t")
    nc.sync.dma_start(cfb[:], d_cfb[:])
    mo, free_mo = tc.tile([P, 1152], BF16, name="mo_t")
    nc.sync.dma_start(mo[:], d_mo[:])
    ones_f = cfb[:, 0:128]
    kqb = cfb[:, 128:144]
    pbc = cfb[:, 144:152]
    b1c = cfb[:, 152:184]
    b2c = cfb[:, 184:192]
    eps_c = cfb[:, 192:193]
    zero_c = cfb[:, 193:194]
    masks = mo[:, 0:1024]  # diagonal-block mask, doubled
    ones_b = mo[:, 1023:1024]  # mask col 511 copy = all-ones column
    invC_row = mo[0:1, 1024:1152]  # [1,128] row of 1/C

    def ones_row_bf(a, w=64):
        # [1, w] bf16 all-ones at partition base 32a (mask rows are ones
        # for cols >= partition index, so the 512-w..512 window is all-ones
        # for rows 0/32/64)
        return mo[32 * a:32 * a + 1, 512 - w:512]

    # ---- allocation stack (LIFO lifetimes) ----
    x2_all, free_x2 = tc.tile([P, 8192], F32, name="x2_all")
    y_all, free_y = tc.tile([P, 8192], BF16, name="y_all")
    h1, free_h1 = tc.tile([P, 8192], BF16, name="h1")

    # ---- layernorm helpers ----
    def ln_eng(i):
        # ~2:1 split DVE:GpSimd (DVE is ~2x faster at elementwise)
        return nc.gpsimd if i in (2, 4, 6) else nc.vector

    def ln_stats_tile(psS, i, src):
        """Accumulate Σx (f32 mm) and Σx² (bf16 mm) for c-tile i."""
        sq = lnp.tile([P, 1024], BF16, tag="sq", name="sq")
        ln_eng(i).tensor_mul(sq[:], src, src)
        for c in range(2):
            nc.tensor.matmul(psS[c][0:1, :], lhsT=cfb[:, 0:1],
                             rhs=src[:, ts(c, 512)],
                             start=(i == 0), stop=(i == 7))
            nc.tensor.matmul(psS[c][32:33, :], lhsT=ones_b[:, 0:1],
                             rhs=sq[:, ts(c, 512)], start=(i == 0), stop=(i == 7),
                             tile_position=(0, 32))

    def ln_tail(psS, hname):
        """Broadcast-first LN tail: Σx/Σx² rows -> (1/C)-scaled 128-row
        broadcasts -> full-width mean/var/rsqrt (no single-lane ladder)."""
        bc, free_bc = tc.tile([P, 2048], F32, name=hname + "_bc")
        srows, free_srows = tc.tile([1, 2048], BF16, name=hname + "_srows")
        nc.scalar.copy(srows[0:1, 0:512], psS[0][0:1, :])
        nc.scalar.copy(srows[0:1, 512:1024], psS[1][0:1, :])
        nc.scalar.copy(srows[0:1, 1024:1536], psS[0][32:33, :])
        nc.scalar.copy(srows[0:1, 1536:2048], psS[1][32:33, :])
        pm = pa()
        pxx = pa()
        for c in range(2):
            nc.tensor.matmul(pm[:, ts(c, 512)], lhsT=invC_row,
                             rhs=srows[0:1, ts(c, 512)], start=True, stop=True)
            nc.tensor.matmul(pxx[:, ts(c, 512)], lhsT=invC_row,
                             rhs=srows[0:1, 1024 + 512 * c:1536 + 512 * c],
                             start=True, stop=True)
        t2 = tmpp.tile([P, 1024], F32, tag="t1", name=hname + "_t2")
        t3 = tmpp.tile([P, 1024], F32, tag="t1", name=hname + "_t3")
        nc.scalar.copy(bc[:, 0:1024], pm[:])
        nc.vector.tensor_mul(t2[:], bc[:, 0:1024], bc[:, 0:1024])
        nc.vector.scalar_tensor_tensor(t3[:], t2[:], -1.0, pxx[:],
                                       ALU.mult, ALU.add)
        act_raw(nc, bc[:, 1024:2048], t3[:], AF.Rsqrt, bias=LN_EPS)
        free_srows()
        return bc, free_bc

    def ln_norm_tile(bc, i, src, dst):
        eng = ln_eng(i)
        t1 = tmpp.tile([P, 1024], F32, tag="t1", name="t1")
        eng.tensor_sub(t1[:], src, bc[:, 0:1024])
        eng.tensor_mul(dst, t1[:], bc[:, 1024:2048])

    # ---- LN1 (x cached in SBUF across stats+normalize) ----
    lnx, free_lnx = tc.tile([P, 8192], F32, name="lnx")
    for i in range(8):
        nc.sync.dma_start(lnx[:, ts(i, 1024)], d_xT[ts(i, 128), :])
    # PE warm-up: dummy matmul burst into a scratch psum during the x-DMA
    # wait so the HAM clock gate reaches 8/8 before the real work starts
    pw_ = pa()
    for _w in range(10):
        nc.tensor.matmul(pw_[:, 0:512], lhsT=ones_b[:, 0:1].broadcast(1, 128)
                         if False else mo[:, 0:128],
                         rhs=mo[:, 0:512], start=(_w == 0), stop=(_w == 9))
    psS1 = [pb(33) for _ in range(2)]
    for i in range(8):
        ln_stats_tile(psS1, i, lnx[:, ts(i, 1024)])
    bc1, free_bc1 = ln_tail(psS1, "h1")
    for i in range(8):
        ln_norm_tile(bc1, i, lnx[:, ts(i, 1024)], h1[:, ts(i, 1024)])
    free_bc1()
    free_lnx()
    if dbg:
        nc.sync.dma_start(dbg["h1"][:], h1[:])

    # ---- v projection (token-major, fused ones column per head) ----
    k_all, free_k = tc.tile([P, 8192], BF16, name="k_all")
    q_all, free_q = tc.tile([P, 8192], BF16, name="q_all")
    v_all, free_v = tc.tile([P, 8320], BF16, name="v_all")
    wv_all, free_wv = tc.tile([P, 8192], BF16, name="wv_all")
    wv_v = d_wv.rearrange("(ct p) o -> p ct o", p=128)
    for c in range(8):
        nc.sync.dma_start(wv_all[:, ts(c, 1024)], wv_v[:, c, :])
    v_view = v_all[:].rearrange("p (a c) -> p a c", c=65)
    nc.vector.memset(v_view[:, :, 64:65], 1.0)
    v_hview = v_all[:].rearrange("p (jt h c) -> p jt h c", jt=8, c=65)
    for jt in range(8):
        psv = pa()
        for c in range(8):
            lhs = h1[:, 1024 * c + 128 * jt:1024 * c + 128 * jt + 128]
            for half in range(2):
                nc.tensor.matmul(psv[:, ts(half, 512)], lhsT=lhs,
                                 rhs=wv_all[:, 1024 * c + 512 * half:1024 * c + 512 * half + 512],
                                 start=(c == 0), stop=(c == 7))
        nc.scalar.copy(v_hview[:, jt, :, 0:64],
                       psv[:].rearrange("p (h c) -> p h c", c=64))
    free_wv()

    # ---- attention helpers ----
    y2_all, free_y2 = tc.tile([P, 8192], BF16, name="y2_all")
    e_buf, free_e = tc.tile([P, 4096], BF16, name="e_buf")
    e_rot = [0]

    def e_slot():
        i = e_rot[0] % 4
        e_rot[0] += 1
        return e_buf[:, 1024 * i:1024 * i + 1024]

    def normalize_quarter(q4, zstash):
        # y2 = y' * (1/Z) broadcast; two broadcasts share one psA tile
        pzt = [None]

        def pz_half(idx):
            if idx % 2 == 0:
                pzt[0] = pa()
            return pzt[0][0:64, 512 * (idx % 2):512 * (idx % 2) + 512]

        idx = 0
        zi_t = tmpp.tile([65, 1536], BF16, tag="zi", name="zi")
        act_raw(nc, zi_t[:], zstash[:], AF.Reciprocal)
        for b in range(3):
            zi = zi_t[:, 512 * b:512 * b + 512]
            for a in range(3):
                li = 3 * b + a
                if li >= 8:
                    break
                hh = 4 * q4 + li // 2
                ch = li % 2
                j, m2 = hh // 2, hh % 2
                r = 64 * m2
                col = 1024 * j + 512 * ch
                pz = pz_half(idx)
                idx += 1
                nc.tensor.matmul(pz, lhsT=ones_row_bf(a),
                                 rhs=zi[32 * a:32 * a + 1, :], start=True, stop=True)
                nc.vector.tensor_mul(y2_all[r:r + 64, col:col + 512], pz,
                                     y_all[r:r + 64, col:col + 512])

    zstashes = {}

    def attn_quarter(q4):
        # 8 Z-row slots per quarter: partition base 32a, col block 512b
        zstash = zpool.tile([65, 1536], F32, tag="zs", name=f"zstash{q4}")
        # only rows 0/32/64 carry Z; zero the rest so the batched reciprocal
        # reads initialized memory (CoreSim requirement, free on GpSimd)
        nc.gpsimd.memset(zstash[:], 1.0)
        zstashes[q4] = zstash
        for j in (2 * q4, 2 * q4 + 1):
            for ch in range(2):
                ntk = 4 if ch == 0 else 8
                py = [pb(), pb()]
                qcol = 1024 * j + 512 * ch
                for jt in range(ntk):
                    pcol = 1024 * j + 128 * jt
                    m = jt - 4 * ch
                    # diagonal trim: tq columns < 128*m are fully masked; skip
                    o = 128 * m if m > 0 else 0
                    ps_ = pa()
                    for m2 in range(2):
                        r = 64 * m2
                        nc.tensor.matmul(ps_[:, 512 * m2 + o:512 * m2 + 512],
                                         lhsT=k_all[r:r + 64, pcol:pcol + 128],
                                         rhs=q_all[r:r + 64, qcol + o:qcol + 512],
                                         start=True, stop=True)
                    et_t = e_slot()
                    et = et_t.rearrange("p (h c) -> p h c", c=512)
                    ps_v = ps_[:].rearrange("p (h c) -> p h c", c=512)
                    nc.scalar.activation(et[:, :, o:512], ps_v[:, :, o:512],
                                         AF.Exp, bias=zero_c, scale=0.125)
                    if m >= 0:
                        # in-place diagonal-block mask (exact-alias DVE, probed safe)
                        nc.vector.tensor_mul(
                            et[:, :, o:o + 128], et[:, :, o:o + 128],
                            masks[:].rearrange("p (h c) -> p h c", c=512)[:, 0:2, 0:128])
                    for m2 in range(2):
                        hh = 2 * j + m2
                        nc.tensor.matmul(
                            py[m2][:, o:512],
                            lhsT=v_all[:, 1040 * jt + 65 * hh:1040 * jt + 65 * hh + 65],
                            rhs=et[:, m2, o:512],
                            start=(jt == 0), stop=(jt == ntk - 1))
                for m2 in range(2):
                    hh = 2 * j + m2
                    r = 64 * m2
                    col = 1024 * j + 512 * ch
                    li = (hh % 4) * 2 + ch
                    a, b = li % 3, li // 3
                    if m2 == 0:
                        nc.scalar.copy(y_all[r:r + 64, col:col + 512],
                                       py[m2][0:64, :])
                    else:
                        nc.vector.tensor_copy(y_all[r:r + 64, col:col + 512],
                                              py[m2][0:64, :])
                    zdst = zstash[32 * a:32 * a + 1, 512 * b:512 * b + 512]
                    nc.vector.tensor_copy(zdst, py[m2][64:65, :])

    # ---- kq pairs interleaved with attention quarters ----
    wkq_v = d_wkq.rearrange("(ct p) o -> p ct o", p=128)
    for g4 in range(4):
        wg = wp.tile([P, 4096], BF16, tag="wg", name="wg")
        wgv = wg[:].rearrange("p (ct o) -> p ct o", o=512)
        for cc in range(4):
            nc.sync.dma_start(wgv[:, 2 * cc:2 * cc + 2, :],
                              wkq_v[:, 2 * cc:2 * cc + 2, ts(g4, 512)])
        for hl in range(4):
            hh = 4 * g4 + hl
            j, r = hh // 2, (hh % 2) * 64
            pp = pa()
            for c in range(8):
                for ch in range(2):
                    nc.tensor.matmul(pp[:, ts(ch, 512)], lhsT=wgv[:, c, ts(hl, 128)],
                                     rhs=h1[:, 1024 * c + 512 * ch:1024 * c + 512 * ch + 512],
                                     start=(c == 0), stop=(c == 7))
            col = 1024 * j
            nc.vector.tensor_scalar_add(k_all[r:r + 64, col:col + 1024], pp[0:64, :],
                                        kqb[0:64, hh:hh + 1])
            nc.vector.tensor_scalar_add(q_all[r:r + 64, col:col + 1024], pp[64:128, :],
                                        kqb[64:128, hh:hh + 1])
        if g4 > 0:
            attn_quarter(g4 - 1)
        if g4 > 1:
            normalize_quarter(g4 - 2, zstashes.pop(g4 - 2))
    attn_quarter(3)
    normalize_quarter(2, zstashes.pop(2))
    normalize_quarter(3, zstashes.pop(3))
    free_e()
    if dbg:
        nc.sync.dma_start(dbg["k"][:], k_all[:])
        nc.sync.dma_start(dbg["q"][:], q_all[:])
        nc.sync.dma_start(dbg["v"][:], v_all[:])
        nc.sync.dma_start(dbg["y"][:], y2_all[:])

    # ---- proj + residual, LN2 stats interleaved ----
    psS2 = [pb(33) for _ in range(2)]
    pw_v = d_pw.rearrange("(ct p) o -> p ct o", p=128)
    for jg in range(2):
        wg = wp.tile([P, 4096], BF16, tag="wg", name="wgp")
        wgv = wg[:].rearrange("p (ct o) -> p ct o", o=512)
        for cc in range(4):
            nc.sync.dma_start(wgv[:, 2 * cc:2 * cc + 2, :],
                              pw_v[:, 2 * cc:2 * cc + 2, ts(jg, 512)])
        for jl in range(4):
            jj = 4 * jg + jl
            pp = pa()
            for c in range(8):
                for ch in range(2):
                    nc.tensor.matmul(pp[:, ts(ch, 512)], lhsT=wgv[:, c, ts(jl, 128)],
                                     rhs=y2_all[:, 1024 * c + 512 * ch:1024 * c + 512 * ch + 512],
                                     start=(c == 0), stop=(c == 7))
            xr = tmpp.tile([P, 1024], F32, tag="xs", name="xr")
            nc.sync.dma_start(xr[:], d_xT[ts(jj, 128), :])
            nc.vector.scalar_tensor_tensor(
                x2_all[:, ts(jj, 1024)], pp[:], pbc[:, jj:jj + 1],
                xr[:], ALU.add, ALU.add)
            ln_stats_tile(psS2, jj, x2_all[:, ts(jj, 1024)])
    free_y2()
    free_v()
    free_q()
    free_k()
    free_h1()
    free_y()
    if dbg:
        nc.sync.dma_start(dbg["x2"][:], x2_all[:])

    # ---- LN2 tail + fc1 + gelu (g allocated below h2 so h2 frees first) ----
    g_all, free_g = tc.tile([P, 32768], BF16, name="g_all")
    h2, free_h2 = tc.tile([P, 8192], BF16, name="h2")
    bc2, free_bc2 = ln_tail(psS2, "h2")
    for i in range(8):
        ln_norm_tile(bc2, i, x2_all[:, ts(i, 1024)], h2[:, ts(i, 1024)])
    free_bc2()
    w1_v = d_w1.rearrange("(ct p) o -> p ct o", p=128)
    for og in range(8):
        wg = wp.tile([P, 4096], BF16, tag="wg", name="wg1")
        wgv = wg[:].rearrange("p (ct o) -> p ct o", o=512)
        for cc in range(4):
            nc.sync.dma_start(wgv[:, 2 * cc:2 * cc + 2, :],
                              w1_v[:, 2 * cc:2 * cc + 2, ts(og, 512)])
        for ol in range(4):
            oo = 4 * og + ol
            pp = pa()
            for c in range(8):
                for ch in range(2):
                    nc.tensor.matmul(pp[:, ts(ch, 512)], lhsT=wgv[:, c, ts(ol, 128)],
                                     rhs=h2[:, 1024 * c + 512 * ch:1024 * c + 512 * ch + 512],
                                     start=(c == 0), stop=(c == 7))
            nc.scalar.activation(g_all[:, ts(oo, 1024)], pp[:],
                                 AF.Gelu, bias=b1c[:, oo:oo + 1])
    free_h2()
    if dbg:
        nc.sync.dma_start(dbg["g"][:], g_all[:])

    # ---- fc2 + residual -> out ----
    w2_v = d_w2.rearrange("(kk p) o -> p kk o", p=128)
    for j in range(8):
        wg = wp.tile([P, 4096], BF16, tag="wg", name="wg2")
        wgv = wg[:].rearrange("p (kk o) -> p kk o", o=128)
        for kg in range(4):
            nc.sync.dma_start(wgv[:, 8 * kg:8 * kg + 8, :],
                              w2_v[:, 8 * kg:8 * kg + 8, ts(j, 128)])
        pp = pa()
        for kk in range(32):
            for ch in range(2):
                nc.tensor.matmul(pp[:, ts(ch, 512)], lhsT=wgv[:, kk, :],
                                 rhs=g_all[:, 1024 * kk + 512 * ch:1024 * kk + 512 * ch + 512],
                                 start=(kk == 0), stop=(kk == 31))
        x3 = outp.tile([P, 1024], F32, tag="x3", name="x3")
        nc.vector.scalar_tensor_tensor(
            x3[:], pp[:], b2c[:, j:j + 1],
            x2_all[:, ts(j, 1024)], ALU.add, ALU.add)
        nc.sync.dma_start(d_out[ts(j, 128), :], x3[:])
    free_g()
    free_x2()
    free_mo()
    free_cfb()


# ---------------- host side ----------------

def prep_inputs(inputs):
    """Build the per-core in_maps from the full problem inputs."""
    f32 = np.float32
    bf16 = ml_dtypes.bfloat16
    x = np.asarray(inputs["x"], f32)
    kqv_w = np.asarray(inputs["kqv_w"], f32)
    kqv_b = np.asarray(inputs["kqv_b"], f32)
    proj_w = np.asarray(inputs["proj_w"], f32)
    proj_b = np.asarray(inputs["proj_b"], f32)
    fc1_w = np.asarray(inputs["fc1_w"], f32)
    fc1_b = np.asarray(inputs["fc1_b"], f32)
    fc2_w = np.asarray(inputs["fc2_w"], f32)
    fc2_b = np.asarray(inputs["fc2_b"], f32)

    wT = np.ascontiguousarray(kqv_w.T).reshape(C, H, 192)
    wkq = np.ascontiguousarray(wT[:, :, :128].reshape(C, 2048)).astype(bf16)
    wv = np.ascontiguousarray(wT[:, :, 128:].reshape(C, 1024)).astype(bf16)
    pw = np.ascontiguousarray(proj_w.T).astype(bf16)
    w1 = np.ascontiguousarray(fc1_w.T).astype(bf16)
    w2 = np.ascontiguousarray(fc2_w.T).astype(bf16)

    kq_b = kqv_b.reshape(H, 192)[:, :128].T  # [128, 16]
    v_b = kqv_b.reshape(H, 192)[:, 128:].reshape(C)
    pb = proj_b + proj_w.astype(np.float64) @ v_b.astype(np.float64)
    pb_col = pb.astype(f32).reshape(8, 128).T  # [128, 8]
    b1_col = fc1_b.reshape(32, 128).T  # [128, 32]
    b2_col = fc2_b.reshape(8, 128).T  # [128, 8]

    cfb = np.zeros((P, 200), f32)
    cfb[:, 0:128] = 1.0
    cfb[:, 128:144] = kq_b
    cfb[:, 144:152] = pb_col
    cfb[:, 152:184] = b1_col
    cfb[:, 184:192] = b2_col
    cfb[:, 192] = LN_EPS

    mo = np.zeros((P, 1024), np.float32)
    pcol = np.arange(128)[:, None]
    frow = np.arange(512)[None, :]
    blk = (frow >= pcol).astype(np.float32)
    mo[:, 0:512] = blk
    mo[:, 512:1024] = blk
    mo = np.concatenate([mo, np.full((P, 128), 1.0 / C, np.float32)], axis=1)
    mo = mo.astype(bf16)

    xT = np.ascontiguousarray(x.transpose(0, 2, 1)).astype(f32)  # [B, C, T]

    shared = dict(wkq=wkq, wv=wv, pw=pw, w1=w1, w2=w2, cfb=cfb, mo=mo)
    in_maps = [dict(shared, xT=xT[b]) for b in range(NB)]
    return in_maps


_CACHE = {}


def get_nc(debug=False):
    key = bool(debug)
    if key not in _CACHE:
        _CACHE[key] = build_nc(debug=debug)
    return _CACHE[key]


def run(inputs, debug=False, trace=False):
    nc = get_nc(debug=debug)
    in_maps = prep_inputs(inputs)
    res = bass_utils.run_bass_kernel_spmd(nc, in_maps, core_ids=list(range(NB)),
                                          trace=trace)
    return res


def kernel(**inputs):
    res = run(inputs, debug=False, trace=False)
    out = np.stack([np.asarray(res.results[b]["out"]).T for b in range(NB)])
    return np.ascontiguousarray(out.astype(np.float32))
